# revision 1
# baseline (speedup 1.0000x reference)
"""DiagonalAffine kernel for Trainium2: y = x * A_diag + B.

x: (262144, 512) f32. Data-parallel over 8 NeuronCores: each core gets a
contiguous slice of 32768 rows; the tiny A_diag/B vectors are replicated
across the 128 SBUF partitions (pre-broadcast on host) so the on-chip
compute is two unit-stride fp32 tensor_tensor ops per tile on the
vector engine, with the A/B operands read through a step-0 broadcast AP.

Per-core streaming loop: DMA-in a [128, F_ROWS*512] tile (rows packed so
each partition holds F_ROWS consecutive rows = contiguous DRAM runs),
DVE multiply by A, DVE add B (in place), DMA-out. Loads go on the SP
HWDGE ring (nc.sync), stores on the ACT ring (nc.scalar) so the two
directions don't head-of-line block each other.
"""

import os
import sys

import numpy as np

_TRN_REPO = "/opt/trn_rl_repo"
if os.path.isdir(_TRN_REPO) and _TRN_REPO not in sys.path:
    sys.path.insert(0, _TRN_REPO)

N, D = 262144, 512
N_CORES = 8
ROWS_PER_CORE = N // N_CORES  # 32768

P = 128              # SBUF partitions
F_ROWS = int(os.environ.get("K_F_ROWS", "4"))   # rows of x per partition per tile
TILE_FREE = F_ROWS * D
ROWS_PER_TILE = P * F_ROWS
X_BUFS = int(os.environ.get("K_BUFS", "8"))
AB_BCAST = os.environ.get("K_AB_BCAST", "1") == "1"

_BUILD_CACHE: dict = {}


def _build(rows_per_core: int):
    """Build the per-core Bass program (identical on all cores)."""
    import concourse.bacc as bacc
    import concourse.tile as tile
    from concourse import mybir

    f32 = mybir.dt.float32
    n_tiles = rows_per_core // ROWS_PER_TILE
    assert n_tiles * ROWS_PER_TILE == rows_per_core

    ab_free = D if AB_BCAST else TILE_FREE

    nc = bacc.Bacc("TRN2", debug=False, num_devices=N_CORES)
    x_in = nc.dram_tensor("x", [rows_per_core, D], f32, kind="ExternalInput")
    a_in = nc.dram_tensor("a_rep", [P, ab_free], f32, kind="ExternalInput")
    b_in = nc.dram_tensor("b_rep", [P, ab_free], f32, kind="ExternalInput")
    y_out = nc.dram_tensor("y", [rows_per_core, D], f32, kind="ExternalOutput")

    xv = x_in[:, :].rearrange("(t p f) d -> t p (f d)", p=P, f=F_ROWS)
    yv = y_out[:, :].rearrange("(t p f) d -> t p (f d)", p=P, f=F_ROWS)

    with tile.TileContext(nc) as tc:
        with (
            tc.tile_pool(name="const", bufs=1) as cpool,
            tc.tile_pool(name="xp", bufs=X_BUFS) as xpool,
        ):
            a_t = cpool.tile([P, ab_free], f32, tag="a")
            nc.sync.dma_start(out=a_t[:], in_=a_in[:, :])
            b_t = cpool.tile([P, ab_free], f32, tag="b")
            nc.sync.dma_start(out=b_t[:], in_=b_in[:, :])

            if AB_BCAST:
                a_ap = a_t[:, :].unsqueeze(1).to_broadcast((P, F_ROWS, D))
                b_ap = b_t[:, :].unsqueeze(1).to_broadcast((P, F_ROWS, D))
            else:
                a_ap = a_t[:, :]
                b_ap = b_t[:, :]

            for t in range(n_tiles):
                xt = xpool.tile([P, TILE_FREE], f32)
                nc.sync.dma_start(out=xt[:], in_=xv[t])
                if AB_BCAST:
                    x_ap = xt[:, :].rearrange("p (r d) -> p r d", d=D)
                else:
                    x_ap = xt[:, :]
                nc.vector.tensor_mul(x_ap, x_ap, a_ap)
                nc.vector.tensor_add(x_ap, x_ap, b_ap)
                nc.scalar.dma_start(out=yv[t], in_=xt[:])
    nc.finalize()
    return nc


def _get_nc(rows_per_core: int):
    nc = _BUILD_CACHE.get(rows_per_core)
    if nc is None:
        nc = _build(rows_per_core)
        _BUILD_CACHE[rows_per_core] = nc
    return nc


# test.py reads this after a traced call for HW timing info.
LAST_RESULTS = None


def kernel(
    x: np.ndarray,
    A_diag: np.ndarray,
    B: np.ndarray,
    trace: bool = False,
    **trace_kwargs,
) -> np.ndarray:
    from concourse.bass_utils import run_bass_kernel_spmd

    global LAST_RESULTS

    x = np.ascontiguousarray(np.asarray(x, dtype=np.float32))
    A_diag = np.asarray(A_diag, dtype=np.float32).reshape(D)
    B = np.asarray(B, dtype=np.float32).reshape(D)
    assert x.shape == (N, D)

    reps = 1 if AB_BCAST else F_ROWS
    a_rep = np.ascontiguousarray(np.tile(A_diag, (P, reps)))
    b_rep = np.ascontiguousarray(np.tile(B, (P, reps)))

    in_maps = [
        {
            "x": x[i * ROWS_PER_CORE : (i + 1) * ROWS_PER_CORE],
            "a_rep": a_rep,
            "b_rep": b_rep,
        }
        for i in range(N_CORES)
    ]

    nc = _get_nc(ROWS_PER_CORE)
    res = run_bass_kernel_spmd(
        nc, in_maps, list(range(N_CORES)), trace=trace, **trace_kwargs
    )
    LAST_RESULTS = res
    out = np.concatenate([r["y"] for r in res.results], axis=0)
    return out.astype(np.float32, copy=False)


if __name__ == "__main__":
    xs = np.random.randn(N, D).astype(np.float32)
    ad = np.random.randn(D).astype(np.float32)
    bs = np.random.randn(D).astype(np.float32)
    y = kernel(xs, ad, bs)
    ref = xs * ad + bs
    err = np.max(np.abs(y - ref)) / (np.max(np.abs(ref)) + 1e-12)
    print("max rel err:", err)

